# revision 18
# baseline (speedup 1.0000x reference)
"""Trainium2 Bass kernel for modality-routed (CogVLM-style) attention, v2.

Contract: kernel(**inputs) takes FULL unsharded numpy inputs (as produced by
the reference's setup_inputs) and returns the FULL [2048, 4096] fp32 output.

Sharding: tensor-parallel over heads. Core r owns heads 4r..4r+3:
  - qkv weights column-sharded; q/k/v all computed in transposed [dim, token]
    orientation via stationary weight tiles (12 uniform m-blocks per expert);
    v is then flipped to natural [token, dim] per 128-token tile with DMA
    transposes (bf16 XBAR path), no PE/PSUM cost.
  - dense weights row-sharded; each core emits a partial output in transposed
    [4096, 2048] orientation, summed + transposed on the host.

Everything between HBM input streams and the final output write stays in
SBUF (no DRAM round-trips for q/k/v/attn). All streams are bf16 (PE rate for
bf16 == fp32r here, but DMA bytes halve and SBUF residency becomes possible);
PSUM accumulation stays fp32. The 1/sqrt(D) score scale is folded into the
softmax exp's activation scale, RoPE rotate-half is one PE matmul with a
signed permutation matrix + two vector muls + one gpsimd add.

Emission interleaves token-segment QKV with attention chunks so the PE
stream never drains at phase boundaries:
  V-seg QKV -> attn c0 | L1 QKV -> attn c1 | L2 -> c2 | L3 -> c3 -> dense.
Expert routing is free: vision tokens are rows 0..575, so expert choice is
just which weight tile streams in for a given token chunk; the vision qkv
bias is a per-partition activation bias on the PSUM evacuation.
"""

import sys

import numpy as np

if "/opt/trn_rl_repo" not in sys.path:
    sys.path.insert(0, "/opt/trn_rl_repo")

import concourse.bass as bass  # noqa: E402,F401
import concourse.tile as tile  # noqa: E402
from concourse import bacc, mybir  # noqa: E402
from concourse.bass_utils import run_bass_kernel_spmd  # noqa: E402

S = 2048
HID = 4096
H = 32
D = 128
NCORES = 8
HPC = H // NCORES          # heads per core = 4
NV = 576                   # vision tokens occupy rows [0, NV)
NKT = HID // 128           # 32 K-tiles
NM = 3 * HPC               # 12 m-blocks (4 q, 4 k, 4 v)
QKSCALE = 1.0 / float(np.sqrt(D))

F32 = mybir.dt.float32
BF = mybir.dt.bfloat16

# token segments (start, end, expert): attention chunk c emits after seg c
SEGS = [(0, 576, 0), (576, 1088, 1), (1088, 1600, 1), (1600, 2048, 1)]
# dense token chunks with expert routing
DCH = [(0, 512, 0), (512, 576, 0), (576, 1088, 1), (1088, 1600, 1),
       (1600, 2048, 1)]

_CACHE = {}


def _chunks(s0, s1):
    out = []
    c = s0
    while c < s1:
        n = min(s1, c + 512)
        out.append((c, n))
        c = n
    return out


def _build():
    nc = bacc.Bacc("TRN2", target_bir_lowering=False, debug=False,
                   num_devices=NCORES)
    dti = nc.dram_tensor
    hs_d = dti("hs", [128, NKT * S], BF, kind="ExternalInput").ap()
    w_d = dti("w", [128, 2 * NM * NKT * 128], BF, kind="ExternalInput").ap()
    wd_d = dti("wd", [128, 2 * 32 * HPC * 128], BF, kind="ExternalInput").ap()
    cos_d = dti("cos", [128, S], BF, kind="ExternalInput").ap()
    sin_d = dti("sin", [128, S], BF, kind="ExternalInput").ap()
    mask_d = dti("mask", [128, 4 * 512], BF, kind="ExternalInput").ap()
    rm_d = dti("rm", [D, D], BF, kind="ExternalInput").ap()
    ones_d = dti("ones", [128, 1], BF, kind="ExternalInput").ap()
    bias_d = dti("bias", [128, NM], F32, kind="ExternalInput").ap()
    rcp_d = dti("rcp", [16, 512], F32).ap()             # broadcast bounce
    out_d = dti("outT", [HID, S], BF, kind="ExternalOutput").ap()

    with tile.TileContext(nc) as tc:
        with tc.tile_pool(name="glob", bufs=1) as glob:
            cos_t = glob.tile([128, S], BF)
            sin_t = glob.tile([128, S], BF)
            mask_t = glob.tile([128, 4, 512], BF)
            rm_t = glob.tile([D, D], BF)
            ones_t = glob.tile([128, 1], BF)
            bias_t = glob.tile([128, NM], F32)

            qT = [glob.tile([128, S], BF, name=f"qT{h}") for h in range(HPC)]
            kT = [glob.tile([128, S], BF, name=f"kT{h}") for h in range(HPC)]
            vT = [glob.tile([128, S], BF, name=f"vT{h}") for h in range(HPC)]
            v_sb = [glob.tile([128, 16, 128], BF, name=f"v{h}")
                    for h in range(HPC)]
            attnT = [glob.tile([128, S], BF, name=f"attnT{h}")
                     for h in range(HPC)]

            with tc.tile_pool(name="hsp", bufs=2) as hs_pool, \
                 tc.tile_pool(name="wp", bufs=2) as w_pool, \
                 tc.tile_pool(name="evp", bufs=3) as ev_pool, \
                 tc.tile_pool(name="pbp", bufs=3) as pb_pool, \
                 tc.tile_pool(name="accp", bufs=2) as acc_pool, \
                 tc.tile_pool(name="smp", bufs=2) as sm_pool, \
                 tc.tile_pool(name="mmps", bufs=3, space="PSUM") as mm_ps, \
                 tc.tile_pool(name="scps", bufs=2, space="PSUM") as sc_ps, \
                 tc.tile_pool(name="spps", bufs=1, space="PSUM") as sp_ps:

                # first segment's streams go first so the PE starts ASAP;
                # constants (needed a few us in) load behind them. Segment 0
                # runs chunk (512,576) first, so its hs slice loads first.
                wt0 = w_pool.tile([128, NKT, 128], BF, tag="w")
                b0 = 8 * NKT * 128
                for q in range(4):
                    nc.sync.dma_start(
                        out=wt0[:, 8 * q:8 * (q + 1), :],
                        in_=w_d[:, b0 + 1024 * q:b0 + 1024 * (q + 1)])
                hst0 = hs_pool.tile([128, NKT, 576], BF, tag="hs")
                for kt in range(NKT):
                    nc.sync.dma_start(
                        out=hst0[:, kt, 512:576],
                        in_=hs_d[:, kt * S + 512:kt * S + 576])
                nc.sync.dma_start(out=bias_t[:], in_=bias_d[:])
                nc.sync.dma_start(out=rm_t[:], in_=rm_d[:])
                wt1 = w_pool.tile([128, NKT, 128], BF, tag="w")
                nc.sync.dma_start(out=wt1[:],
                                  in_=w_d[:, 9 * NKT * 128:10 * NKT * 128])
                nc.sync.dma_start(out=cos_t[:, :576], in_=cos_d[:, :576])
                nc.sync.dma_start(out=sin_t[:, :576], in_=sin_d[:, :576])
                for kt in range(NKT):
                    nc.sync.dma_start(
                        out=hst0[:, kt, :512],
                        in_=hs_d[:, kt * S:kt * S + 512])
                nc.sync.dma_start(out=ones_t[:], in_=ones_d[:])
                nc.sync.dma_start(out=mask_t[:], in_=mask_d[:])
                nc.sync.dma_start(out=cos_t[:, 576:], in_=cos_d[:, 576:])
                nc.sync.dma_start(out=sin_t[:, 576:], in_=sin_d[:, 576:])
                wpre = {8: wt0, 9: wt1}

                # v blocks (m 8..11) first so the attention chunk's v tiles
                # transpose early; q/k follow
                M_ORDER = [8, 9, 10, 11, 0, 1, 2, 3, 4, 5, 6, 7]

                for si, (s0, s1, e) in enumerate(SEGS):
                    sw = s1 - s0
                    if si == 0:
                        hst = hst0
                        seg_chunks = [(512, 576), (0, 512)]
                    else:
                        hst = hs_pool.tile([128, NKT, 576], BF, tag="hs")
                        for kt in range(NKT):
                            nc.sync.dma_start(
                                out=hst[:, kt, :sw],
                                in_=hs_d[:, kt * S + s0:kt * S + s1])
                        seg_chunks = _chunks(s0, s1)
                    rope_pend = []

                    def rope_flush(item):
                        (pt, qk_sb, m, c0, c1) = item
                        w2 = c1 - c0
                        rot = mm_ps.tile([128, 512], F32, tag="mm")
                        nc.tensor.matmul(rot[:, :w2], rm_t[:],
                                         qk_sb[:, :w2],
                                         start=True, stop=True)
                        prod = ev_pool.tile([128, 512], BF, tag="prod")
                        nc.vector.tensor_mul(prod[:, :w2], qk_sb[:, :w2],
                                             cos_t[:, c0:c1])
                        rp = ev_pool.tile([128, 512], BF, tag="rp")
                        nc.vector.tensor_mul(rp[:, :w2], rot[:, :w2],
                                             sin_t[:, c0:c1])
                        tgt = qT[m] if m < HPC else kT[m - HPC]
                        nc.gpsimd.tensor_add(tgt[:, c0:c1], prod[:, :w2],
                                             rp[:, :w2])

                    for mi, m in enumerate(M_ORDER):
                        if si == 0 and m in wpre:
                            wt = wpre[m]
                        else:
                            wt = w_pool.tile([128, NKT, 128], BF, tag="w")
                            base = (e * NM + m) * NKT * 128
                            nc.sync.dma_start(
                                out=wt[:], in_=w_d[:, base:base + NKT * 128])
                        for (c0, c1) in seg_chunks:
                            w2 = c1 - c0
                            o0 = c0 - s0
                            pt = mm_ps.tile([128, 512], F32, tag="mm")
                            for kt in range(NKT):
                                nc.tensor.matmul(pt[:, :w2], wt[:, kt, :],
                                                 hst[:, kt, o0:o0 + w2],
                                                 start=(kt == 0),
                                                 stop=(kt == NKT - 1))
                            if m < 2 * HPC:
                                qk_sb = ev_pool.tile([128, 512], BF,
                                                     tag="qksb")
                                if e == 0:
                                    nc.scalar.activation(
                                        out=qk_sb[:, :w2], in_=pt[:, :w2],
                                        func=mybir.ActivationFunctionType
                                        .Identity,
                                        bias=bias_t[:, m:m + 1], scale=1.0)
                                else:
                                    nc.scalar.activation(
                                        out=qk_sb[:, :w2], in_=pt[:, :w2],
                                        func=mybir.ActivationFunctionType
                                        .Copy, scale=1.0)
                                rope_pend.append((pt, qk_sb, m, c0, c1))
                                if len(rope_pend) > 1:
                                    rope_flush(rope_pend.pop(0))
                            else:
                                mv = m - 2 * HPC
                                if e == 0:
                                    nc.scalar.activation(
                                        out=vT[mv][:, c0:c1], in_=pt[:, :w2],
                                        func=mybir.ActivationFunctionType
                                        .Identity,
                                        bias=bias_t[:, m:m + 1], scale=1.0)
                                else:
                                    nc.scalar.activation(
                                        out=vT[mv][:, c0:c1], in_=pt[:, :w2],
                                        func=mybir.ActivationFunctionType
                                        .Copy, scale=1.0)
                    for item in rope_pend:
                        rope_flush(item)

                    # transpose this chunk's v tiles (deps already resolved)
                    for h in range(HPC):
                        for jt in range(4 * si, 4 * si + 4):
                            nc.sync.dma_start_transpose(
                                out=v_sb[h][:, jt, :],
                                in_=vT[h][:, 128 * jt:128 * (jt + 1)])

                    # ---- attention chunk c (queries 512c .. 512c+512)
                    # cross-head software pipeline: each head's drain hides
                    # behind the next head's score matmuls. Softmax sums
                    # accumulate on the vector engine (acc += probs tiles);
                    # one ones-matmul per (h,c) does the partition reduce.
                    c = si
                    q0 = 512 * c
                    ng = 2 * (c + 1)          # groups of 2 key-tiles
                    # diagonal (masked) groups first: their exp->mask
                    # latency hides behind later scps; drain is mask-free
                    order = [2 * c, 2 * c + 1] + list(range(0, 2 * c))
                    seq = [(h, g, gi == 0, gi == ng - 1)
                           for h in range(HPC)
                           for gi, g in enumerate(order)]
                    ap_t = {}
                    acc_t = {}

                    def finish_head(h, c=c, q0=q0):
                        sp = sp_ps.tile([1, 512], F32, tag="sp")
                        nc.tensor.matmul(sp[:], ones_t[:], acc_t[h][:],
                                         start=True, stop=True)
                        hc = 4 * c + h
                        rc = sm_pool.tile([1, 512], F32, tag="rc")
                        nc.vector.reciprocal(rc[:], sp[:])
                        # bounce via the SWDGE (gpsimd) queue: keeps the
                        # blocking round-trip off the SP stream so segment
                        # prefetches flow during attention
                        nc.gpsimd.dma_start(out=rcp_d[hc:hc + 1, :],
                                            in_=rc[:])
                        rb = sm_pool.tile([128, 512], F32, tag="rb")
                        nc.gpsimd.dma_start(
                            out=rb[:],
                            in_=rcp_d[hc:hc + 1, :].to_broadcast((128, 512)))
                        nc.vector.tensor_mul(attnT[h][:, q0:q0 + 512],
                                             ap_t[h][:], rb[:])

                    def flush(h, g, first, last, pb):
                        if first:
                            ap_t[h] = mm_ps.tile([128, 512], F32, tag="mm",
                                                 name=f"apk{h}")
                        for jj in range(2):
                            j = 2 * g + jj
                            nc.tensor.matmul(ap_t[h][:], v_sb[h][:, j, :],
                                             pb[:, jj, :],
                                             start=(first and jj == 0),
                                             stop=(last and jj == 1))
                        if last:
                            finish_head(h)

                    pend = []
                    for (h, g, first, last) in seq:
                        scp = sc_ps.tile([128, 2, 512], F32, tag="sc")
                        for jj in range(2):
                            j = 2 * g + jj
                            nc.tensor.matmul(
                                scp[:, jj, :],
                                kT[h][:, 128 * j:128 * (j + 1)],
                                qT[h][:, q0:q0 + 512],
                                start=True, stop=True)
                        pb = pb_pool.tile([128, 2, 512], BF, tag="pb")
                        nc.scalar.activation(
                            out=pb[:], in_=scp[:],
                            func=mybir.ActivationFunctionType.Exp,
                            scale=QKSCALE)
                        if g >= 2 * c:
                            r = 2 * (g - 2 * c)
                            nc.gpsimd.tensor_mul(pb[:], pb[:],
                                                 mask_t[:, r:r + 2, :])
                        if first:
                            acc = acc_pool.tile([128, 512], BF, tag="acc")
                            acc_t[h] = acc
                            nc.vector.tensor_add(acc[:], pb[:, 0, :],
                                                 pb[:, 1, :])
                        else:
                            acc = acc_t[h]
                            nc.vector.tensor_add(acc[:], acc[:], pb[:, 0, :])
                            nc.vector.tensor_add(acc[:], acc[:], pb[:, 1, :])
                        pend.append((h, g, first, last, pb))
                        if len(pend) > 1:
                            flush(*pend.pop(0))
                    for it in pend:
                        flush(*it)

            # ---------------- dense phase ----------------
            with tc.tile_pool(name="wdp", bufs=4) as wd_pool, \
                 tc.tile_pool(name="oep", bufs=4) as oe_pool, \
                 tc.tile_pool(name="dnps", bufs=4, space="PSUM") as dn_ps:
                evac_eng = [lambda o, i: nc.scalar.activation(
                                out=o, in_=i,
                                func=mybir.ActivationFunctionType.Copy,
                                scale=1.0),
                            nc.vector.tensor_copy]
                ei = 0
                for o in range(32):
                    wde = []
                    for e in range(2):
                        wdt = wd_pool.tile([128, HPC, 128], BF, tag="wd")
                        base = (e * 32 + o) * HPC * 128
                        nc.sync.dma_start(
                            out=wdt[:], in_=wd_d[:, base:base + HPC * 128])
                        wde.append(wdt)
                    for (t0, t1, e) in DCH:
                        w2 = t1 - t0
                        po = dn_ps.tile([128, 512], F32, tag="po")
                        for hh in range(HPC):
                            nc.tensor.matmul(po[:, :w2], wde[e][:, hh, :],
                                             attnT[hh][:, t0:t1],
                                             start=(hh == 0),
                                             stop=(hh == HPC - 1))
                        oe = oe_pool.tile([128, 512], BF, tag="oe")
                        evac_eng[ei % 2](oe[:, :w2], po[:, :w2])
                        ei += 1
                        nc.sync.dma_start(
                            out=out_d[128 * o:128 * (o + 1), t0:t1],
                            in_=oe[:, :w2])
    nc.compile()
    return nc


def _prep_inputs(inputs):
    import ml_dtypes
    bf = ml_dtypes.bfloat16

    hs = np.asarray(inputs["hidden_states"], np.float32)
    cos = np.asarray(inputs["cos"], np.float32)
    sin = np.asarray(inputs["sin"], np.float32)
    vi = np.asarray(inputs["vision_indices"]).ravel()
    li = np.asarray(inputs["language_indices"]).ravel()
    assert vi.size == NV and np.array_equal(vi, np.arange(NV)) and \
        np.array_equal(li, np.arange(NV, S)), "unsupported index layout"

    # hs tiled [128, (kt, t)]
    hs_t = np.ascontiguousarray(
        hs.T.reshape(NKT, 128, S).transpose(1, 0, 2).reshape(128, NKT * S)
    ).astype(bf)

    cos_t = np.ascontiguousarray(cos.T).astype(bf)
    sin_t = np.ascontiguousarray(sin.T).astype(bf)

    rm = np.zeros((D, D), np.float32)
    for d in range(64):
        rm[d + 64, d] = -1.0
        rm[d, d + 64] = 1.0
    rm = rm.astype(bf)

    # mask[p, r, q'] = 1 iff 128 r + p <= q'
    p = np.arange(128)[:, None, None]
    r = np.arange(4)[None, :, None]
    q = np.arange(512)[None, None, :]
    mask = (128 * r + p <= q).astype(np.float32).reshape(128, 4 * 512)
    mask = mask.astype(bf)

    ones = np.ones((128, 1), np.float32).astype(bf)

    b = np.asarray(inputs["vision_qkv_b"], np.float32)
    Wqkv = np.stack([np.asarray(inputs["vision_qkv_w"], np.float32),
                     np.asarray(inputs["lang_qkv_w"], np.float32)])
    Wd = np.stack([np.asarray(inputs["vision_dense_w"], np.float32),
                   np.asarray(inputs["lang_dense_w"], np.float32)])

    in_maps = []
    for rr in range(NCORES):
        q0 = 512 * rr
        cols = np.r_[q0:q0 + 512, HID + q0:HID + q0 + 512,
                     2 * HID + q0:2 * HID + q0 + 512]
        # w tiled [128, (e, m, kt, c)]
        wc = Wqkv[:, :, cols]                                # [2, 4096, 1536]
        w_t = np.ascontiguousarray(
            wc.reshape(2, NKT, 128, NM, 128)
              .transpose(2, 0, 3, 1, 4)
              .reshape(128, 2 * NM * NKT * 128)).astype(bf)
        # wd tiled [128, (e, o, hh, c)]
        wdc = Wd[:, q0:q0 + 512, :]                          # [2, 512, 4096]
        wd_t = np.ascontiguousarray(
            wdc.reshape(2, HPC, 128, 32, 128)
               .transpose(2, 0, 3, 1, 4)
               .reshape(128, 2 * 32 * HPC * 128)).astype(bf)
        bias_t = np.ascontiguousarray(
            b[cols].reshape(NM, 128).T).astype(np.float32)
        in_maps.append({
            "hs": hs_t, "w": w_t, "wd": wd_t,
            "cos": cos_t, "sin": sin_t, "mask": mask, "rm": rm,
            "ones": ones, "bias": bias_t,
        })
    return in_maps


def kernel(**inputs):
    if "nc" not in _CACHE:
        _CACHE["nc"] = _build()
    nc = _CACHE["nc"]
    in_maps = _prep_inputs(inputs)
    res = run_bass_kernel_spmd(nc, in_maps, list(range(NCORES)),
                               **_CACHE.get("run_kwargs", {}))
    _CACHE["last_results"] = res
    out = np.zeros((HID, S), np.float32)
    for r in range(NCORES):
        out += res.results[r]["outT"].astype(np.float32)
    return np.ascontiguousarray(out.T)


# revision 19
# speedup vs baseline: 1.1332x; 1.1332x over previous
"""Trainium2 Bass kernel for modality-routed (CogVLM-style) attention, v2.

Contract: kernel(**inputs) takes FULL unsharded numpy inputs (as produced by
the reference's setup_inputs) and returns the FULL [2048, 4096] fp32 output.

Sharding: tensor-parallel over heads. Core r owns heads 4r..4r+3:
  - qkv weights column-sharded; q/k/v all computed in transposed [dim, token]
    orientation via stationary weight tiles (12 uniform m-blocks per expert);
    v is then flipped to natural [token, dim] per 128-token tile with DMA
    transposes (bf16 XBAR path), no PE/PSUM cost.
  - dense weights row-sharded; each core emits a partial output in transposed
    [4096, 2048] orientation, summed + transposed on the host.

Everything between HBM input streams and the final output write stays in
SBUF (no DRAM round-trips for q/k/v/attn). All streams are bf16 (PE rate for
bf16 == fp32r here, but DMA bytes halve and SBUF residency becomes possible);
PSUM accumulation stays fp32. The 1/sqrt(D) score scale is folded into the
softmax exp's activation scale, RoPE rotate-half is one PE matmul with a
signed permutation matrix + two vector muls + one gpsimd add.

Emission interleaves token-segment QKV with attention chunks so the PE
stream never drains at phase boundaries:
  V-seg QKV -> attn c0 | L1 QKV -> attn c1 | L2 -> c2 | L3 -> c3 -> dense.
Expert routing is free: vision tokens are rows 0..575, so expert choice is
just which weight tile streams in for a given token chunk; the vision qkv
bias is a per-partition activation bias on the PSUM evacuation.
"""

import sys

import numpy as np

if "/opt/trn_rl_repo" not in sys.path:
    sys.path.insert(0, "/opt/trn_rl_repo")

import concourse.bass as bass  # noqa: E402,F401
import concourse.tile as tile  # noqa: E402
from concourse import bacc, mybir  # noqa: E402
from concourse.bass_utils import run_bass_kernel_spmd  # noqa: E402

S = 2048
HID = 4096
H = 32
D = 128
NCORES = 8
HPC = H // NCORES          # heads per core = 4
NV = 576                   # vision tokens occupy rows [0, NV)
NKT = HID // 128           # 32 K-tiles
NM = 3 * HPC               # 12 m-blocks (4 q, 4 k, 4 v)
QKSCALE = 1.0 / float(np.sqrt(D))

F32 = mybir.dt.float32
BF = mybir.dt.bfloat16

# token segments (start, end, expert): attention chunk c emits after seg c
SEGS = [(0, 576, 0), (576, 1088, 1), (1088, 1600, 1), (1600, 2048, 1)]
# dense token chunks with expert routing
DCH = [(0, 512, 0), (512, 576, 0), (576, 1088, 1), (1088, 1600, 1),
       (1600, 2048, 1)]

_CACHE = {}


def _chunks(s0, s1):
    out = []
    c = s0
    while c < s1:
        n = min(s1, c + 512)
        out.append((c, n))
        c = n
    return out


def _build():
    nc = bacc.Bacc("TRN2", target_bir_lowering=False, debug=False,
                   num_devices=NCORES)
    dti = nc.dram_tensor
    hs_d = dti("hs", [128, NKT * S], BF, kind="ExternalInput").ap()
    w_d = dti("w", [128, 2 * NM * NKT * 128], BF, kind="ExternalInput").ap()
    wd_d = dti("wd", [128, 2 * 32 * HPC * 128], BF, kind="ExternalInput").ap()
    cos_d = dti("cos", [128, S], BF, kind="ExternalInput").ap()
    sin_d = dti("sin", [128, S], BF, kind="ExternalInput").ap()
    mask_d = dti("mask", [128, 4 * 512], BF, kind="ExternalInput").ap()
    rm_d = dti("rm", [D, D], BF, kind="ExternalInput").ap()
    ones_d = dti("ones", [128, 1], BF, kind="ExternalInput").ap()
    bias_d = dti("bias", [128, NM], F32, kind="ExternalInput").ap()
    rcp_d = dti("rcp", [16, 512], F32).ap()             # broadcast bounce
    out_d = dti("outT", [HID, S], BF, kind="ExternalOutput").ap()

    with tile.TileContext(nc) as tc:
        with tc.tile_pool(name="glob", bufs=1) as glob:
            cos_t = glob.tile([128, S], BF)
            sin_t = glob.tile([128, S], BF)
            mask_t = glob.tile([128, 4, 512], BF)
            rm_t = glob.tile([D, D], BF)
            ones_t = glob.tile([128, 1], BF)
            bias_t = glob.tile([128, NM], F32)

            qT = [glob.tile([128, S], BF, name=f"qT{h}") for h in range(HPC)]
            kT = [glob.tile([128, S], BF, name=f"kT{h}") for h in range(HPC)]
            vT = [glob.tile([128, S], BF, name=f"vT{h}") for h in range(HPC)]
            v_sb = [glob.tile([128, 16, 128], BF, name=f"v{h}")
                    for h in range(HPC)]
            attnT = [glob.tile([128, S], BF, name=f"attnT{h}")
                     for h in range(HPC)]

            with tc.tile_pool(name="hsp", bufs=2) as hs_pool, \
                 tc.tile_pool(name="wp", bufs=2) as w_pool, \
                 tc.tile_pool(name="evp", bufs=3) as ev_pool, \
                 tc.tile_pool(name="pbp", bufs=3) as pb_pool, \
                 tc.tile_pool(name="accp", bufs=2) as acc_pool, \
                 tc.tile_pool(name="smp", bufs=2) as sm_pool, \
                 tc.tile_pool(name="mmps", bufs=3, space="PSUM") as mm_ps, \
                 tc.tile_pool(name="scps", bufs=2, space="PSUM") as sc_ps, \
                 tc.tile_pool(name="spps", bufs=1, space="PSUM") as sp_ps:

                # first segment's streams go first so the PE starts ASAP;
                # constants (needed a few us in) load behind them. Segment 0
                # runs chunk (512,576) first, so its hs slice loads first.
                wt0 = w_pool.tile([128, NKT, 128], BF, tag="w")
                b0 = 8 * NKT * 128
                for q in range(4):
                    nc.sync.dma_start(
                        out=wt0[:, 8 * q:8 * (q + 1), :],
                        in_=w_d[:, b0 + 1024 * q:b0 + 1024 * (q + 1)])
                hst0 = hs_pool.tile([128, NKT, 576], BF, tag="hs")
                for kt in range(NKT):
                    nc.sync.dma_start(
                        out=hst0[:, kt, 512:576],
                        in_=hs_d[:, kt * S + 512:kt * S + 576])
                nc.sync.dma_start(out=bias_t[:], in_=bias_d[:])
                nc.sync.dma_start(out=rm_t[:], in_=rm_d[:])
                wt1 = w_pool.tile([128, NKT, 128], BF, tag="w")
                b1 = 9 * NKT * 128
                for q in range(4):
                    nc.sync.dma_start(
                        out=wt1[:, 8 * q:8 * (q + 1), :],
                        in_=w_d[:, b1 + 1024 * q:b1 + 1024 * (q + 1)])
                nc.sync.dma_start(out=cos_t[:, :576], in_=cos_d[:, :576])
                nc.sync.dma_start(out=sin_t[:, :576], in_=sin_d[:, :576])
                for kt in range(NKT):
                    nc.sync.dma_start(
                        out=hst0[:, kt, :512],
                        in_=hs_d[:, kt * S:kt * S + 512])
                nc.sync.dma_start(out=ones_t[:], in_=ones_d[:])
                nc.sync.dma_start(out=mask_t[:], in_=mask_d[:])
                nc.sync.dma_start(out=cos_t[:, 576:], in_=cos_d[:, 576:])
                nc.sync.dma_start(out=sin_t[:, 576:], in_=sin_d[:, 576:])
                wpre = {8: wt0, 9: wt1}

                # v blocks (m 8..11) first so the attention chunk's v tiles
                # transpose early; q/k follow
                M_ORDER = [8, 9, 10, 11, 0, 1, 2, 3, 4, 5, 6, 7]

                hs_next = hst0
                for si, (s0, s1, e) in enumerate(SEGS):
                    sw = s1 - s0
                    hst = hs_next
                    if si == 0:
                        seg_chunks = [(512, 576), (0, 512)]
                    else:
                        seg_chunks = _chunks(s0, s1)
                    rope_pend = []

                    def rope_flush(item):
                        (pt, qk_sb, m, c0, c1) = item
                        w2 = c1 - c0
                        rot = mm_ps.tile([128, 512], F32, tag="mm")
                        nc.tensor.matmul(rot[:, :w2], rm_t[:],
                                         qk_sb[:, :w2],
                                         start=True, stop=True)
                        prod = ev_pool.tile([128, 512], BF, tag="prod")
                        nc.vector.tensor_mul(prod[:, :w2], qk_sb[:, :w2],
                                             cos_t[:, c0:c1])
                        rp = ev_pool.tile([128, 512], BF, tag="rp")
                        nc.vector.tensor_mul(rp[:, :w2], rot[:, :w2],
                                             sin_t[:, c0:c1])
                        tgt = qT[m] if m < HPC else kT[m - HPC]
                        nc.gpsimd.tensor_add(tgt[:, c0:c1], prod[:, :w2],
                                             rp[:, :w2])

                    for mi, m in enumerate(M_ORDER):
                        if si == 0 and m in wpre:
                            wt = wpre[m]
                        else:
                            wt = w_pool.tile([128, NKT, 128], BF, tag="w")
                            base = (e * NM + m) * NKT * 128
                            for q in range(4):
                                nc.sync.dma_start(
                                    out=wt[:, 8 * q:8 * (q + 1), :],
                                    in_=w_d[:, base + 1024 * q:
                                            base + 1024 * (q + 1)])
                        for (c0, c1) in seg_chunks:
                            w2 = c1 - c0
                            o0 = c0 - s0
                            pt = mm_ps.tile([128, 512], F32, tag="mm")
                            for kt in range(NKT):
                                nc.tensor.matmul(pt[:, :w2], wt[:, kt, :],
                                                 hst[:, kt, o0:o0 + w2],
                                                 start=(kt == 0),
                                                 stop=(kt == NKT - 1))
                            if m < 2 * HPC:
                                qk_sb = ev_pool.tile([128, 512], BF,
                                                     tag="qksb")
                                if e == 0:
                                    nc.scalar.activation(
                                        out=qk_sb[:, :w2], in_=pt[:, :w2],
                                        func=mybir.ActivationFunctionType
                                        .Identity,
                                        bias=bias_t[:, m:m + 1], scale=1.0)
                                else:
                                    nc.scalar.activation(
                                        out=qk_sb[:, :w2], in_=pt[:, :w2],
                                        func=mybir.ActivationFunctionType
                                        .Copy, scale=1.0)
                                rope_pend.append((pt, qk_sb, m, c0, c1))
                                if len(rope_pend) > 1:
                                    rope_flush(rope_pend.pop(0))
                            else:
                                mv = m - 2 * HPC
                                if e == 0:
                                    nc.scalar.activation(
                                        out=vT[mv][:, c0:c1], in_=pt[:, :w2],
                                        func=mybir.ActivationFunctionType
                                        .Identity,
                                        bias=bias_t[:, m:m + 1], scale=1.0)
                                else:
                                    nc.scalar.activation(
                                        out=vT[mv][:, c0:c1], in_=pt[:, :w2],
                                        func=mybir.ActivationFunctionType
                                        .Copy, scale=1.0)
                    for item in rope_pend:
                        rope_flush(item)

                    # transpose this chunk's v tiles (deps already resolved)
                    for h in range(HPC):
                        for jt in range(4 * si, 4 * si + 4):
                            nc.sync.dma_start_transpose(
                                out=v_sb[h][:, jt, :],
                                in_=vT[h][:, 128 * jt:128 * (jt + 1)])

                    # prefetch next segment's hidden states ahead of the
                    # blocking softmax bounce DMAs in the SP stream
                    if si + 1 < len(SEGS):
                        n0, n1, _ = SEGS[si + 1]
                        hs_next = hs_pool.tile([128, NKT, 576], BF, tag="hs")
                        for kt in range(NKT):
                            nc.sync.dma_start(
                                out=hs_next[:, kt, :n1 - n0],
                                in_=hs_d[:, kt * S + n0:kt * S + n1])

                    # ---- attention chunk c (queries 512c .. 512c+512)
                    # cross-head software pipeline: each head's drain hides
                    # behind the next head's score matmuls. Softmax sums
                    # accumulate on the vector engine (acc += probs tiles);
                    # one ones-matmul per (h,c) does the partition reduce.
                    c = si
                    q0 = 512 * c
                    ng = 2 * (c + 1)          # groups of 2 key-tiles
                    # diagonal (masked) groups first: their exp->mask
                    # latency hides behind later scps; drain is mask-free
                    order = [2 * c, 2 * c + 1] + list(range(0, 2 * c))
                    seq = [(h, g, gi == 0, gi == ng - 1)
                           for h in range(HPC)
                           for gi, g in enumerate(order)]
                    ap_t = {}
                    acc_t = {}

                    def finish_head(h, c=c, q0=q0):
                        sp = sp_ps.tile([1, 512], F32, tag="sp")
                        nc.tensor.matmul(sp[:], ones_t[:], acc_t[h][:],
                                         start=True, stop=True)
                        hc = 4 * c + h
                        rc = sm_pool.tile([1, 512], F32, tag="rc")
                        nc.vector.reciprocal(rc[:], sp[:])
                        nc.sync.dma_start(out=rcp_d[hc:hc + 1, :],
                                           in_=rc[:])
                        rb = sm_pool.tile([128, 512], F32, tag="rb")
                        nc.sync.dma_start(
                            out=rb[:],
                            in_=rcp_d[hc:hc + 1, :].to_broadcast((128, 512)))
                        nc.vector.tensor_mul(attnT[h][:, q0:q0 + 512],
                                             ap_t[h][:], rb[:])

                    def flush(h, g, first, last, pb):
                        if first:
                            ap_t[h] = mm_ps.tile([128, 512], F32, tag="mm",
                                                 name=f"apk{h}")
                        for jj in range(2):
                            j = 2 * g + jj
                            nc.tensor.matmul(ap_t[h][:], v_sb[h][:, j, :],
                                             pb[:, jj, :],
                                             start=(first and jj == 0),
                                             stop=(last and jj == 1))
                        if last:
                            finish_head(h)

                    pend = []
                    for (h, g, first, last) in seq:
                        scp = sc_ps.tile([128, 2, 512], F32, tag="sc")
                        for jj in range(2):
                            j = 2 * g + jj
                            nc.tensor.matmul(
                                scp[:, jj, :],
                                kT[h][:, 128 * j:128 * (j + 1)],
                                qT[h][:, q0:q0 + 512],
                                start=True, stop=True)
                        pb = pb_pool.tile([128, 2, 512], BF, tag="pb")
                        nc.scalar.activation(
                            out=pb[:], in_=scp[:],
                            func=mybir.ActivationFunctionType.Exp,
                            scale=QKSCALE)
                        if g >= 2 * c:
                            r = 2 * (g - 2 * c)
                            nc.gpsimd.tensor_mul(pb[:], pb[:],
                                                 mask_t[:, r:r + 2, :])
                        if first:
                            acc = acc_pool.tile([128, 512], BF, tag="acc")
                            acc_t[h] = acc
                            nc.vector.tensor_add(acc[:], pb[:, 0, :],
                                                 pb[:, 1, :])
                        else:
                            acc = acc_t[h]
                            nc.vector.tensor_add(acc[:], acc[:], pb[:, 0, :])
                            nc.vector.tensor_add(acc[:], acc[:], pb[:, 1, :])
                        pend.append((h, g, first, last, pb))
                        if len(pend) > 1:
                            flush(*pend.pop(0))
                    for it in pend:
                        flush(*it)

            # ---------------- dense phase ----------------
            with tc.tile_pool(name="wdp", bufs=4) as wd_pool, \
                 tc.tile_pool(name="oep", bufs=4) as oe_pool, \
                 tc.tile_pool(name="dnps", bufs=4, space="PSUM") as dn_ps:
                evac_eng = [lambda o, i: nc.scalar.activation(
                                out=o, in_=i,
                                func=mybir.ActivationFunctionType.Copy,
                                scale=1.0),
                            nc.vector.tensor_copy]
                ei = 0
                for o in range(32):
                    wde = []
                    for e in range(2):
                        wdt = wd_pool.tile([128, HPC, 128], BF, tag="wd")
                        base = (e * 32 + o) * HPC * 128
                        nc.sync.dma_start(
                            out=wdt[:], in_=wd_d[:, base:base + HPC * 128])
                        wde.append(wdt)
                    for (t0, t1, e) in DCH:
                        w2 = t1 - t0
                        po = dn_ps.tile([128, 512], F32, tag="po")
                        for hh in range(HPC):
                            nc.tensor.matmul(po[:, :w2], wde[e][:, hh, :],
                                             attnT[hh][:, t0:t1],
                                             start=(hh == 0),
                                             stop=(hh == HPC - 1))
                        oe = oe_pool.tile([128, 512], BF, tag="oe")
                        evac_eng[ei % 2](oe[:, :w2], po[:, :w2])
                        ei += 1
                        nc.sync.dma_start(
                            out=out_d[128 * o:128 * (o + 1), t0:t1],
                            in_=oe[:, :w2])
    nc.compile()
    return nc


def _prep_inputs(inputs):
    import ml_dtypes
    bf = ml_dtypes.bfloat16

    hs = np.asarray(inputs["hidden_states"], np.float32)
    cos = np.asarray(inputs["cos"], np.float32)
    sin = np.asarray(inputs["sin"], np.float32)
    vi = np.asarray(inputs["vision_indices"]).ravel()
    li = np.asarray(inputs["language_indices"]).ravel()
    assert vi.size == NV and np.array_equal(vi, np.arange(NV)) and \
        np.array_equal(li, np.arange(NV, S)), "unsupported index layout"

    # hs tiled [128, (kt, t)]
    hs_t = np.ascontiguousarray(
        hs.T.reshape(NKT, 128, S).transpose(1, 0, 2).reshape(128, NKT * S)
    ).astype(bf)

    cos_t = np.ascontiguousarray(cos.T).astype(bf)
    sin_t = np.ascontiguousarray(sin.T).astype(bf)

    rm = np.zeros((D, D), np.float32)
    for d in range(64):
        rm[d + 64, d] = -1.0
        rm[d, d + 64] = 1.0
    rm = rm.astype(bf)

    # mask[p, r, q'] = 1 iff 128 r + p <= q'
    p = np.arange(128)[:, None, None]
    r = np.arange(4)[None, :, None]
    q = np.arange(512)[None, None, :]
    mask = (128 * r + p <= q).astype(np.float32).reshape(128, 4 * 512)
    mask = mask.astype(bf)

    ones = np.ones((128, 1), np.float32).astype(bf)

    b = np.asarray(inputs["vision_qkv_b"], np.float32)
    Wqkv = np.stack([np.asarray(inputs["vision_qkv_w"], np.float32),
                     np.asarray(inputs["lang_qkv_w"], np.float32)])
    Wd = np.stack([np.asarray(inputs["vision_dense_w"], np.float32),
                   np.asarray(inputs["lang_dense_w"], np.float32)])

    in_maps = []
    for rr in range(NCORES):
        q0 = 512 * rr
        cols = np.r_[q0:q0 + 512, HID + q0:HID + q0 + 512,
                     2 * HID + q0:2 * HID + q0 + 512]
        # w tiled [128, (e, m, kt, c)]
        wc = Wqkv[:, :, cols]                                # [2, 4096, 1536]
        w_t = np.ascontiguousarray(
            wc.reshape(2, NKT, 128, NM, 128)
              .transpose(2, 0, 3, 1, 4)
              .reshape(128, 2 * NM * NKT * 128)).astype(bf)
        # wd tiled [128, (e, o, hh, c)]
        wdc = Wd[:, q0:q0 + 512, :]                          # [2, 512, 4096]
        wd_t = np.ascontiguousarray(
            wdc.reshape(2, HPC, 128, 32, 128)
               .transpose(2, 0, 3, 1, 4)
               .reshape(128, 2 * 32 * HPC * 128)).astype(bf)
        bias_t = np.ascontiguousarray(
            b[cols].reshape(NM, 128).T).astype(np.float32)
        in_maps.append({
            "hs": hs_t, "w": w_t, "wd": wd_t,
            "cos": cos_t, "sin": sin_t, "mask": mask, "rm": rm,
            "ones": ones, "bias": bias_t,
        })
    return in_maps


def kernel(**inputs):
    if "nc" not in _CACHE:
        _CACHE["nc"] = _build()
    nc = _CACHE["nc"]
    in_maps = _prep_inputs(inputs)
    res = run_bass_kernel_spmd(nc, in_maps, list(range(NCORES)),
                               **_CACHE.get("run_kwargs", {}))
    _CACHE["last_results"] = res
    out = np.zeros((HID, S), np.float32)
    for r in range(NCORES):
        out += res.results[r]["outT"].astype(np.float32)
    return np.ascontiguousarray(out.T)
